# revision 28
# baseline (speedup 1.0000x reference)
"""Causal self-attention (B=4, T=2048, C=1024, H=16, HD=64) on 8 trn2 cores.

Sharding: core = (batch b, head-group g) with g in {0,1} covering 8 heads each.
Each core computes, for its (b, g):
    QKV projection (its 8 heads' columns of W_attn), causal attention for the
    8 heads, and the partial output projection y_g @ W_proj[g*512:(g+1)*512].
Host sums the two partial projections per batch and adds b_proj.

Numerics: x / W_qk / W_v / Q / K / V / P(=exp S) all bf16 (CPU-simulated
max-rel-err of this scheme is 2.3e-3); PSUM accumulation, biases, Y and the
output projection (f32r) stay fp32.

Per-core schedule (n = 512-col T-chunk, stage n overlaps attention qc=n-1):
  stage 0: QK^T projection groups for n=0 (lhsT=wqk, rhs=x^T chunk; DVE adds
    bias, writes QT/KT bf16), then V projections t=0..3.
  stages 1..3: QK groups for chunk n and V projections t=4n..4n+3 are used as
    PE filler units inside emit_b(qc=n-1, hp) — the attention inner loop is
    ACT(exp)-paced, so fillers slot into the S->exp->AV dependency bubbles
    (2 units after the S prologue, 1 every 3rd kc block).
  stage 4: attention qc=3 with output-projection tiles t=0..11 as fillers;
    trailing t=12..15 output projections close the kernel.
Attention block (qc, hp): S^T[k,q] for the even/odd head of the pair go into
one [128,2,512] PSUM tile (contraction d=64 at partitions 0..63 / 64..127 of
QT/KT); one wide ACT exp (scale=1/8, no max subtraction needed) -> pt bf16;
diagonal blocks are width-reduced and get a fixed lower-triangle mask multiply.
AV: per parity, lhsT=[V|ones] accumulates y into PSUM partitions 0..63 with
the softmax denominator in row 64. Normalize per parity: gpsimd broadcasts
the PSUM sum row, DVE reciprocal, then one DVE multiply straight from PSUM
into YT (even) or a staging tile DMA'd to partitions 64..127 (odd).
DMA: x streams in 4 T-chunks (bf16 halves the traffic); wqk is split in m
pairs ordered (01,45,23,67) so the first QK group starts ~4us in; everything
else is dependency-gated behind the startup-critical set.
"""

import numpy as np

B, T, C, H, HD = 4, 2048, 1024, 16, 64
G = 2              # head groups (tensor parallel)
HG = H // G        # 8 heads per group
GC = HG * HD       # 512 group channels
P = 128
NQC = T // 512     # 4 q-chunks of 512
NKC = T // P       # 16 k-chunks of 128
KO_C = C // P      # 8 contraction chunks for C=1024
KO_G = GC // P     # 4 contraction chunks for GC=512 (= head pairs)

_cache = {}


def _build():
    import concourse.bass as bass
    import concourse.tile as tile
    from concourse import bacc, mybir

    f32 = mybir.dt.float32
    f32r = mybir.dt.float32r
    bf16 = mybir.dt.bfloat16

    nc = bacc.Bacc(name="csa")
    xT = nc.declare_dram_parameter("xT", [P, NQC, KO_C, 512], bf16, isOutput=False)
    wqk = nc.declare_dram_parameter("wqk", [P, 2 * GC // P, KO_C, P], bf16, isOutput=False)
    bqk = nc.declare_dram_parameter("bqk", [P, 2 * GC // P], f32, isOutput=False)
    wv = nc.declare_dram_parameter("wv", [P, KO_C, GC], bf16, isOutput=False)
    bv = nc.declare_dram_parameter("bv", [P, 2, HG // 2, HD], f32, isOutput=False)
    wp = nc.declare_dram_parameter("wp", [P, KO_G, C], f32r, isOutput=False)
    mask = nc.declare_dram_parameter("mask", [P, P], bf16, isOutput=False)
    out = nc.declare_dram_parameter("out", [T, C], f32, isOutput=True)

    from contextlib import ExitStack

    with tile.TileContext(nc) as tc, ExitStack() as ctx:
        singles = ctx.enter_context(tc.tile_pool(name="singles", bufs=1))
        ppool = ctx.enter_context(tc.tile_pool(name="ppool", bufs=3))
        spool = ctx.enter_context(tc.tile_pool(name="spool", bufs=2))
        opool = ctx.enter_context(tc.tile_pool(name="opool", bufs=2))
        pp = ctx.enter_context(tc.tile_pool(name="pp", bufs=2, space="PSUM"))
        ps = ctx.enter_context(tc.tile_pool(name="ps", bufs=2, space="PSUM"))
        py = ctx.enter_context(tc.tile_pool(name="py", bufs=2, space="PSUM"))

        # ---- resident tensors ----
        xbig = singles.tile([P, NQC, KO_C, 512], bf16, tag="xbig")
        QT = singles.tile([P, HG // 2, T], bf16, tag="QT")
        KT = singles.tile([P, HG // 2, T], bf16, tag="KT")
        # V augmented, parity-major: [V | ones] (col 64 = softmax denominator)
        vaug = singles.tile([P, NKC, 2, HG // 2, 65], bf16, tag="vaug")
        YT = singles.tile([P, KO_G, T], f32r, tag="YT")
        wqk_s = singles.tile([P, 2 * GC // P, KO_C, P], bf16, tag="wqk")
        wv_s = singles.tile([P, KO_C, GC], bf16, tag="wv")
        wp_s = singles.tile([P, KO_G, C], f32r, tag="wp")
        bqk_s = singles.tile([P, 2 * GC // P], f32, tag="bqk")
        bv_s = singles.tile([P, 2, HG // 2, HD], f32, tag="bv")
        tri = singles.tile([P, P], bf16, tag="tri")
        ones_sb = singles.tile([P, 1], bf16, tag="ones_sb")

        # ---- DMA emission is just-in-time: emission order == queue priority,
        # and consumers appear to wait on all previously-emitted DMAs, so only
        # the startup-critical set (x chunk 0, wqk, smalls) precedes the first
        # matmul; later loads are emitted right before their first consumers.
        def emit_x(n):
            for h in (0, 1):
                nc.sync.dma_start(
                    out=xbig[:, n, 4 * h:4 * h + 4, :],
                    in_=xT[:, n, 4 * h:4 * h + 4, :],
                )

        emit_x(0)
        # startup-critical weights: first two m-pairs; the rest and all smalls
        # are gated behind them so the first QK groups' data gets full HBM bw
        gate = None
        for m0 in (0, 4):
            gate = nc.sync.dma_start(out=wqk_s[:, m0:m0 + 2], in_=wqk[:, m0:m0 + 2])

        def gated(d):
            tile.add_dep_helper(d.ins, gate.ins, reason="after startup set")
            return d

        for m0 in (2, 6):
            gated(nc.sync.dma_start(out=wqk_s[:, m0:m0 + 2], in_=wqk[:, m0:m0 + 2]))
        gated(nc.sync.dma_start(out=tri[:], in_=mask[:]))
        gated(nc.sync.dma_start(out=bqk_s[:], in_=bqk[:]))
        gated(nc.sync.dma_start(out=bv_s[:], in_=bv[:]))

        nc.vector.memset(ones_sb[:], 1.0)
        for parity in (0, 1):
            nc.vector.tensor_copy(
                out=vaug[:, :, parity, :, 64:65],
                in_=ones_sb[:, :, None, None].to_broadcast(
                    (P, NKC, HG // 2, 1)),
            )

        # ---- emitters ----
        def emit_qk_group(m, n):
            acc = pp.tile([P, 512], f32, tag="pp")
            for ko in range(KO_C):
                nc.tensor.matmul(
                    acc[:],
                    lhsT=wqk_s[:, m, ko, :],
                    rhs=xbig[:, n, ko, :],
                    start=(ko == 0),
                    stop=(ko == KO_C - 1),
                )
            dest = QT if m < 4 else KT
            nc.vector.tensor_tensor(
                dest[:, m % 4, n * 512:(n + 1) * 512],
                acc[:],
                bqk_s[:, m:m + 1].to_broadcast((P, 512)),
                mybir.AluOpType.add,
            )

        def emit_v(t):
            acc = pp.tile([P, 512], f32, tag="pp")
            for ko in range(KO_C):
                nc.tensor.matmul(
                    acc[:],
                    lhsT=xbig[:, t // 4, ko, (t % 4) * P:(t % 4 + 1) * P],
                    rhs=wv_s[:, ko, :],
                    start=(ko == 0),
                    stop=(ko == KO_C - 1),
                )
            accv = acc[:].rearrange("p (two hp d) -> p two hp d", two=2, hp=HG // 2)
            nc.vector.tensor_tensor(
                vaug[:, t, 0, :, 0:64], accv[:, 0], bv_s[:, 0],
                mybir.AluOpType.add,
            )
            nc.vector.tensor_tensor(
                vaug[:, t, 1, :, 0:64], accv[:, 1], bv_s[:, 1],
                mybir.AluOpType.add,
            )

        def emit_c(t, n):
            opsum = pp.tile([P, 512], f32, tag="pp")
            for ko in range(KO_G):
                nc.tensor.matmul(
                    opsum[:],
                    lhsT=YT[:, ko, t * P:(t + 1) * P],
                    rhs=wp_s[:, ko, n * 512:(n + 1) * 512],
                    start=(ko == 0),
                    stop=(ko == KO_G - 1),
                )
            osb = opool.tile([P, 512], f32, tag="osb")
            nc.vector.tensor_copy(out=osb[:], in_=opsum[:])
            nc.sync.dma_start(
                out=out[t * P:(t + 1) * P, n * 512:(n + 1) * 512],
                in_=osb[:],
            )

        def emit_norm(qc, hp, ype, ypo):
            qcols = slice(qc * 512, (qc + 1) * 512)
            # odd first: its chain is longer (staging DMA); sum-row staging on
            # the scalar engine (gpsimd can't read PSUM, DVE is busier).
            # gpsimd partition_broadcast only works with base-partition-0 APs.
            for odd, yp in ((1, ypo), (0, ype)):
                ssum = spool.tile([1, 512], f32, tag="ssum")
                nc.scalar.activation(
                    ssum[:], yp[64:65, :], mybir.ActivationFunctionType.Copy,
                )
                srep = spool.tile([64, 512], f32, tag="srep")
                nc.gpsimd.partition_broadcast(srep[:], ssum[:])
                nc.vector.reciprocal_approx_fast(out=srep[:], in_=srep[:])
                if odd == 0:
                    nc.vector.tensor_tensor(
                        YT[0:64, hp, qcols], yp[0:64, :], srep[:],
                        mybir.AluOpType.mult,
                    )
                else:
                    # DVE lanes can't shift partitions; stage odd at 0..63
                    # and DMA to partitions 64..127
                    ystage = spool.tile([64, 512], f32r, tag="ystage")
                    nc.vector.tensor_tensor(
                        ystage[:], yp[0:64, :], srep[:], mybir.AluOpType.mult,
                    )
                    nc.sync.dma_start(out=YT[64:128, hp, qcols], in_=ystage[:])

        def emit_b(qc, hp, fill):
            nkc = 4 * (qc + 1)
            ype = py.tile([P, 512], f32, tag="py")
            ypo = py.tile([P, 512], f32, tag="py")
            pts = {}

            def emit_s(kc):
                j = kc - 4 * qc
                qo = max(j, 0) * P
                w = 512 - qo
                spsum = ps.tile([P, 2, 512], f32, tag="ps")
                for odd in (0, 1):
                    po = odd * 64
                    nc.tensor.matmul(
                        spsum[:, odd, 0:w],
                        lhsT=KT[po:po + 64, hp, kc * P:(kc + 1) * P],
                        rhs=QT[po:po + 64, hp, qc * 512 + qo:(qc + 1) * 512],
                        start=True,
                        stop=True,
                    )
                pt = ppool.tile([P, 2, 512], bf16, tag="pt")
                nc.scalar.activation(
                    pt[:, :, 0:w], spsum[:, :, 0:w],
                    mybir.ActivationFunctionType.Exp, scale=0.125,
                )
                if j >= 0:
                    nc.vector.tensor_tensor(
                        pt[:, :, 0:P], pt[:, :, 0:P],
                        tri[:, None, :].to_broadcast((P, 2, P)),
                        mybir.AluOpType.mult,
                    )
                pts[kc] = (pt, qo, w)

            def emit_av(kc):
                pt, qo, w = pts.pop(kc)
                for odd, yp in ((0, ype), (1, ypo)):
                    nc.tensor.matmul(
                        yp[0:65, qo:512],
                        lhsT=vaug[:, kc, odd, hp, :],
                        rhs=pt[:, odd, 0:w],
                        start=(kc == 0),
                        stop=(kc == nkc - 1),
                    )

            emit_s(0)
            if nkc > 1:
                emit_s(1)
            fill(2)
            for kc in range(nkc):
                if kc + 2 < nkc:
                    emit_s(kc + 2)
                emit_av(kc)
                if kc % 3 == 2:
                    fill(1)
            emit_norm(qc, hp, ype, ypo)

        class Fillers:
            def __init__(self):
                self.q = []

            def add(self, fn):
                self.q.append(fn)

            def pop(self, n=1):
                for _ in range(n):
                    if self.q:
                        self.q.pop(0)()

            def flush(self):
                while self.q:
                    self.q.pop(0)()

        # ---- schedule ----
        # stage 0: n=0 QK groups + V t=0..3
        for m in (0, 4, 1, 5, 2, 6, 3, 7):
            emit_qk_group(m, 0)
        for h in (0, 1):
            gated(nc.sync.dma_start(out=wv_s[:, 4 * h:4 * h + 4, :],
                                    in_=wv[:, 4 * h:4 * h + 4, :]))
        for t in range(4):
            emit_v(t)
        for n in range(1, NQC):
            for h in (0, 1):
                gated(nc.sync.dma_start(
                    out=xbig[:, n, 4 * h:4 * h + 4, :],
                    in_=xT[:, n, 4 * h:4 * h + 4, :],
                ))
        # stages 1..3: QK chunk n + V t=4n.. as fillers inside attention qc=n-1
        for n in range(1, NQC):
            qc = n - 1
            fillers = Fillers()
            for m in (0, 4, 1, 5, 2, 6, 3, 7):
                fillers.add(lambda m=m, n=n: emit_qk_group(m, n))
            for t in range(4 * n, 4 * n + 4):
                fillers.add(lambda t=t: emit_v(t))
            if n == 3:
                for t in range(0, 4):
                    for nn in (0, 1):
                        fillers.add(lambda t=t, nn=nn: emit_c(t, nn))
            for hp in range(HG // 2):
                emit_b(qc, hp, fillers.pop)
            fillers.flush()
            if n == 1:
                for ko in range(KO_G):
                    nc.sync.dma_start(out=wp_s[:, ko, :], in_=wp[:, ko, :])
        # stage 4: attention qc=3 with output projections t=4..11 as fillers
        fillers = Fillers()
        for t in range(4, 12):
            for nn in (0, 1):
                fillers.add(lambda t=t, nn=nn: emit_c(t, nn))
        for hp in range(HG // 2):
            emit_b(3, hp, fillers.pop)
        fillers.flush()
        # trailing output projections for the last q-chunk
        for t in range(12, 16):
            for nn in (0, 1):
                emit_c(t, nn)
    nc.finalize()
    return nc


def _get_nc():
    if "nc" not in _cache:
        _cache["nc"] = _build()
    return _cache["nc"]


def _prep_inputs(x, W_attn, b_attn, W_proj):
    import ml_dtypes

    bfnp = ml_dtypes.bfloat16
    x = np.asarray(x, np.float32)
    W_attn = np.asarray(W_attn, np.float32)
    b_attn = np.asarray(b_attn, np.float32)
    W_proj = np.asarray(W_proj, np.float32)
    mask = (np.arange(P)[:, None] <= np.arange(P)[None, :]).astype(bfnp)
    in_maps = []
    for b in range(B):
        xTb = np.ascontiguousarray(
            x[b].T.reshape(KO_C, P, NQC, 512).transpose(1, 2, 0, 3).astype(bfnp))
        for g in range(G):
            qs, ks, vs = g * GC, C + g * GC, 2 * C + g * GC
            w2 = np.concatenate([W_attn[:, qs:qs + GC], W_attn[:, ks:ks + GC]], 1)
            # wv columns / bv reordered parity-major: (two, hp, d)
            wv_r = (W_attn[:, vs:vs + GC].reshape(C, HG // 2, 2, HD)
                    .transpose(0, 2, 1, 3).reshape(C, GC))
            bv_r = (b_attn[vs:vs + GC].reshape(HG // 2, 2, HD)
                    .transpose(1, 0, 2))
            # wp rows grouped by head pair: ko chunk = hp, [even d | odd d]
            wp_r = W_proj[g * GC:(g + 1) * GC, :]  # rows already (hp,two,d) order
            in_maps.append({
                "xT": xTb,
                "wqk": np.ascontiguousarray(
                    w2.reshape(KO_C, P, 2 * GC // P, P).transpose(1, 2, 0, 3)
                    .astype(bfnp)),
                "bqk": np.ascontiguousarray(
                    np.concatenate([b_attn[qs:qs + GC], b_attn[ks:ks + GC]])
                    .reshape(2 * GC // P, P).T),
                "wv": np.ascontiguousarray(
                    wv_r.reshape(KO_C, P, GC).transpose(1, 0, 2).astype(bfnp)),
                "bv": np.ascontiguousarray(
                    np.broadcast_to(bv_r, (P, 2, HG // 2, HD))),
                "wp": np.ascontiguousarray(
                    wp_r.reshape(KO_G, P, C).transpose(1, 0, 2)),
                "mask": mask,
            })
    return in_maps


def _run(inputs, trace=False):
    from concourse.bass_utils import run_bass_kernel_spmd

    nc = _get_nc()
    in_maps = _prep_inputs(
        inputs["x"], inputs["W_attn"], inputs["b_attn"], inputs["W_proj"]
    )
    res = run_bass_kernel_spmd(nc, in_maps, list(range(B * G)), trace=trace)
    b_proj = np.asarray(inputs["b_proj"], np.float32)
    outs = [
        res.results[2 * b]["out"] + res.results[2 * b + 1]["out"] + b_proj
        for b in range(B)
    ]
    return np.stack(outs).astype(np.float32), res


def kernel(**inputs):
    return _run(inputs, trace=False)[0]


if __name__ == "__main__":
    rng = np.random.default_rng(0)
    ins = {
        "x": rng.standard_normal((B, T, C), np.float32),
        "W_attn": rng.uniform(-0.03, 0.03, (C, 3 * C)).astype(np.float32),
        "b_attn": rng.uniform(-0.03, 0.03, (3 * C,)).astype(np.float32),
        "W_proj": rng.uniform(-0.03, 0.03, (C, C)).astype(np.float32),
        "b_proj": rng.uniform(-0.03, 0.03, (C,)).astype(np.float32),
    }
    out = kernel(**ins)
    print("ran, out shape", out.shape)


# revision 34
# speedup vs baseline: 1.0241x; 1.0241x over previous
"""Causal self-attention (B=4, T=2048, C=1024, H=16, HD=64) on 8 trn2 cores.

Sharding: core = (batch b, head-group g) with g in {0,1} covering 8 heads each.
Each core computes, for its (b, g):
    QKV projection (its 8 heads' columns of W_attn), causal attention for the
    8 heads, and the partial output projection y_g @ W_proj[g*512:(g+1)*512].
Host sums the two partial projections per batch and adds b_proj.

Numerics: x / W_qk / W_v / Q / K / V / P(=exp S) all bf16 (CPU-simulated
max-rel-err of this scheme is 2.3e-3); PSUM accumulation, biases, Y and the
output projection (f32r) stay fp32.

Per-core schedule (n = 512-col T-chunk, stage n overlaps attention qc=n-1):
  stage 0: QK^T projection groups for n=0 (lhsT=wqk, rhs=x^T chunk; DVE adds
    bias, writes QT/KT bf16), then V projections t=0..3.
  stages 1..3: QK groups for chunk n and V projections t=4n..4n+3 are used as
    PE filler units inside emit_b(qc=n-1, hp) — the attention inner loop is
    ACT(exp)-paced, so fillers slot into the S->exp->AV dependency bubbles
    (2 units after the S prologue, 1 every 3rd kc block).
  stage 4: attention qc=3 with output-projection tiles t=0..11 as fillers;
    trailing t=12..15 output projections close the kernel.
Attention block (qc, hp): S^T[k,q] for the even/odd head of the pair go into
one [128,2,512] PSUM tile (contraction d=64 at partitions 0..63 / 64..127 of
QT/KT); one wide ACT exp (scale=1/8, no max subtraction needed) -> pt bf16;
diagonal blocks are width-reduced and get a fixed lower-triangle mask multiply.
AV: per parity, lhsT=[V|ones] accumulates y into PSUM partitions 0..63 with
the softmax denominator in row 64. Normalize per parity: gpsimd broadcasts
the PSUM sum row, DVE reciprocal, then one DVE multiply straight from PSUM
into YT (even) or a staging tile DMA'd to partitions 64..127 (odd).
DMA: x streams in 4 T-chunks (bf16 halves the traffic); wqk is split in m
pairs ordered (01,45,23,67) so the first QK group starts ~4us in; everything
else is dependency-gated behind the startup-critical set.
"""

import numpy as np

B, T, C, H, HD = 4, 2048, 1024, 16, 64
G = 2              # head groups (tensor parallel)
HG = H // G        # 8 heads per group
GC = HG * HD       # 512 group channels
P = 128
NQC = T // 512     # 4 q-chunks of 512
NKC = T // P       # 16 k-chunks of 128
KO_C = C // P      # 8 contraction chunks for C=1024
KO_G = GC // P     # 4 contraction chunks for GC=512 (= head pairs)

_cache = {}


def _build():
    import concourse.bass as bass
    import concourse.tile as tile
    from concourse import bacc, mybir

    f32 = mybir.dt.float32
    f32r = mybir.dt.float32r
    bf16 = mybir.dt.bfloat16

    nc = bacc.Bacc(name="csa")
    xT = nc.declare_dram_parameter("xT", [P, NQC, KO_C, 512], bf16, isOutput=False)
    wqk = nc.declare_dram_parameter("wqk", [P, 2 * GC // P, KO_C, P], bf16, isOutput=False)
    bqk = nc.declare_dram_parameter("bqk", [P, 2 * GC // P], f32, isOutput=False)
    wv = nc.declare_dram_parameter("wv", [P, KO_C, GC], bf16, isOutput=False)
    bv = nc.declare_dram_parameter("bv", [P, 2, HG // 2, HD], f32, isOutput=False)
    wp = nc.declare_dram_parameter("wp", [P, KO_G, C], f32r, isOutput=False)
    mask = nc.declare_dram_parameter("mask", [P, P], bf16, isOutput=False)
    out = nc.declare_dram_parameter("out", [T, C], f32, isOutput=True)

    from contextlib import ExitStack

    with tile.TileContext(nc) as tc, ExitStack() as ctx:
        singles = ctx.enter_context(tc.tile_pool(name="singles", bufs=1))
        ppool = ctx.enter_context(tc.tile_pool(name="ppool", bufs=4))
        spool = ctx.enter_context(tc.tile_pool(name="spool", bufs=2))
        opool = ctx.enter_context(tc.tile_pool(name="opool", bufs=2))
        pp = ctx.enter_context(tc.tile_pool(name="pp", bufs=2, space="PSUM"))
        ps = ctx.enter_context(tc.tile_pool(name="ps", bufs=2, space="PSUM"))
        py = ctx.enter_context(tc.tile_pool(name="py", bufs=2, space="PSUM"))

        # ---- resident tensors ----
        xbig = singles.tile([P, NQC, KO_C, 512], bf16, tag="xbig")
        QT = singles.tile([P, HG // 2, T], bf16, tag="QT")
        KT = singles.tile([P, HG // 2, T], bf16, tag="KT")
        # V augmented, parity-major: [V | ones] (col 64 = softmax denominator)
        vaug = singles.tile([P, NKC, 2, HG // 2, 65], bf16, tag="vaug")
        YT = singles.tile([P, KO_G, T], f32r, tag="YT")
        wqk_s = singles.tile([P, 2 * GC // P, KO_C, P], bf16, tag="wqk")
        wv_s = singles.tile([P, KO_C, GC], bf16, tag="wv")
        wp_s = singles.tile([P, KO_G, C], f32r, tag="wp")
        bqk_s = singles.tile([P, 2 * GC // P], f32, tag="bqk")
        bv_s = singles.tile([P, 2, HG // 2, HD], f32, tag="bv")
        tri = singles.tile([P, P], bf16, tag="tri")
        ones_sb = singles.tile([P, 1], bf16, tag="ones_sb")

        # ---- DMA emission is just-in-time: emission order == queue priority,
        # and consumers appear to wait on all previously-emitted DMAs, so only
        # the startup-critical set (x chunk 0, wqk, smalls) precedes the first
        # matmul; later loads are emitted right before their first consumers.
        def emit_x(n):
            for h in (0, 1):
                nc.sync.dma_start(
                    out=xbig[:, n, 4 * h:4 * h + 4, :],
                    in_=xT[:, n, 4 * h:4 * h + 4, :],
                )

        emit_x(0)
        for m0 in (0, 4, 2, 6):  # m pairs ordered for first-use order (0,4),(1,5)..
            nc.sync.dma_start(out=wqk_s[:, m0:m0 + 2], in_=wqk[:, m0:m0 + 2])
        nc.sync.dma_start(out=tri[:], in_=mask[:])
        nc.sync.dma_start(out=bqk_s[:], in_=bqk[:])
        nc.sync.dma_start(out=bv_s[:], in_=bv[:])

        nc.vector.memset(ones_sb[:], 1.0)
        for parity in (0, 1):
            nc.vector.tensor_copy(
                out=vaug[:, :, parity, :, 64:65],
                in_=ones_sb[:, :, None, None].to_broadcast(
                    (P, NKC, HG // 2, 1)),
            )

        # ---- emitters ----
        def emit_qk_group(m, n):
            acc = pp.tile([P, 512], f32, tag="pp")
            for ko in range(KO_C):
                nc.tensor.matmul(
                    acc[:],
                    lhsT=wqk_s[:, m, ko, :],
                    rhs=xbig[:, n, ko, :],
                    start=(ko == 0),
                    stop=(ko == KO_C - 1),
                )
            dest = QT if m < 4 else KT
            nc.vector.tensor_tensor(
                dest[:, m % 4, n * 512:(n + 1) * 512],
                acc[:],
                bqk_s[:, m:m + 1].to_broadcast((P, 512)),
                mybir.AluOpType.add,
            )

        def emit_v(t):
            acc = pp.tile([P, 512], f32, tag="pp")
            for ko in range(KO_C):
                nc.tensor.matmul(
                    acc[:],
                    lhsT=xbig[:, t // 4, ko, (t % 4) * P:(t % 4 + 1) * P],
                    rhs=wv_s[:, ko, :],
                    start=(ko == 0),
                    stop=(ko == KO_C - 1),
                )
            accv = acc[:].rearrange("p (two hp d) -> p two hp d", two=2, hp=HG // 2)
            nc.vector.tensor_tensor(
                vaug[:, t, 0, :, 0:64], accv[:, 0], bv_s[:, 0],
                mybir.AluOpType.add,
            )
            nc.vector.tensor_tensor(
                vaug[:, t, 1, :, 0:64], accv[:, 1], bv_s[:, 1],
                mybir.AluOpType.add,
            )

        def emit_c(t, n):
            opsum = pp.tile([P, 512], f32, tag="pp")
            for ko in range(KO_G):
                nc.tensor.matmul(
                    opsum[:],
                    lhsT=YT[:, ko, t * P:(t + 1) * P],
                    rhs=wp_s[:, ko, n * 512:(n + 1) * 512],
                    start=(ko == 0),
                    stop=(ko == KO_G - 1),
                )
            osb = opool.tile([P, 512], f32, tag="osb")
            nc.vector.tensor_copy(out=osb[:], in_=opsum[:])
            nc.sync.dma_start(
                out=out[t * P:(t + 1) * P, n * 512:(n + 1) * 512],
                in_=osb[:],
            )

        def emit_norm(qc, hp, ype, ypo):
            qcols = slice(qc * 512, (qc + 1) * 512)
            # odd first: its chain is longer (staging DMA); sum-row staging on
            # the scalar engine (gpsimd can't read PSUM, DVE is busier).
            # gpsimd partition_broadcast only works with base-partition-0 APs.
            for odd, yp in ((1, ypo), (0, ype)):
                ssum = spool.tile([1, 512], f32, tag="ssum")
                nc.scalar.activation(
                    ssum[:], yp[64:65, :], mybir.ActivationFunctionType.Copy,
                )
                srep = spool.tile([64, 512], f32, tag="srep")
                nc.gpsimd.partition_broadcast(srep[:], ssum[:])
                nc.vector.reciprocal_approx_fast(out=srep[:], in_=srep[:])
                if odd == 0:
                    nc.vector.tensor_tensor(
                        YT[0:64, hp, qcols], yp[0:64, :], srep[:],
                        mybir.AluOpType.mult,
                    )
                else:
                    # DVE lanes can't shift partitions; stage odd at 0..63
                    # and DMA to partitions 64..127
                    ystage = spool.tile([64, 512], f32r, tag="ystage")
                    nc.vector.tensor_tensor(
                        ystage[:], yp[0:64, :], srep[:], mybir.AluOpType.mult,
                    )
                    nc.sync.dma_start(out=YT[64:128, hp, qcols], in_=ystage[:])

        def emit_b(qc, hp, fill):
            nkc = 4 * (qc + 1)
            ype = py.tile([P, 512], f32, tag="py")
            ypo = py.tile([P, 512], f32, tag="py")
            pts = {}

            def emit_s(kc):
                j = kc - 4 * qc
                qo = max(j, 0) * P
                w = 512 - qo
                spsum = ps.tile([P, 2, 512], f32, tag="ps")
                for odd in (0, 1):
                    po = odd * 64
                    nc.tensor.matmul(
                        spsum[:, odd, 0:w],
                        lhsT=KT[po:po + 64, hp, kc * P:(kc + 1) * P],
                        rhs=QT[po:po + 64, hp, qc * 512 + qo:(qc + 1) * 512],
                        start=True,
                        stop=True,
                    )
                pt = ppool.tile([P, 2, 512], bf16, tag="pt")
                nc.scalar.activation(
                    pt[:, :, 0:w], spsum[:, :, 0:w],
                    mybir.ActivationFunctionType.Exp, scale=0.125,
                )
                if j >= 0:
                    nc.vector.tensor_tensor(
                        pt[:, :, 0:P], pt[:, :, 0:P],
                        tri[:, None, :].to_broadcast((P, 2, P)),
                        mybir.AluOpType.mult,
                    )
                pts[kc] = (pt, qo, w)

            def emit_av(kc):
                pt, qo, w = pts.pop(kc)
                for odd, yp in ((1, ypo), (0, ype)):
                    nc.tensor.matmul(
                        yp[0:65, qo:512],
                        lhsT=vaug[:, kc, odd, hp, :],
                        rhs=pt[:, odd, 0:w],
                        start=(kc == 0),
                        stop=(kc == nkc - 1),
                    )

            emit_s(0)
            if nkc > 1:
                emit_s(1)
            fill(2)
            for kc in range(nkc):
                if kc + 2 < nkc:
                    emit_s(kc + 2)
                emit_av(kc)
                if kc % 3 == 2:
                    fill(1)
            emit_norm(qc, hp, ype, ypo)

        class Fillers:
            def __init__(self):
                self.q = []

            def add(self, fn):
                self.q.append(fn)

            def pop(self, n=1):
                for _ in range(n):
                    if self.q:
                        self.q.pop(0)()

            def flush(self):
                while self.q:
                    self.q.pop(0)()

        # ---- schedule ----
        # stage 0: n=0 QK groups + V t=0..3
        for m in (0, 4, 1, 5, 2, 6, 3, 7):
            emit_qk_group(m, 0)
        for h in (0, 1):
            nc.sync.dma_start(out=wv_s[:, 4 * h:4 * h + 4, :],
                              in_=wv[:, 4 * h:4 * h + 4, :])
        for t in range(4):
            emit_v(t)
        for n in range(1, NQC):
            emit_x(n)
        # stages 1..3: QK chunk n + V t=4n.. as fillers inside attention qc=n-1
        for n in range(1, NQC):
            qc = n - 1
            fillers = Fillers()
            for m in (0, 4, 1, 5, 2, 6, 3, 7):
                fillers.add(lambda m=m, n=n: emit_qk_group(m, n))
            if n < 3:
                for t in range(4 * n, 4 * n + 4):
                    fillers.add(lambda t=t: emit_v(t))
            else:
                # V t=12..15 defers to stage 4 (its filler-starved stretch)
                for t in range(0, 4):
                    for nn in (0, 1):
                        fillers.add(lambda t=t, nn=nn: emit_c(t, nn))
            for hp in range(HG // 2):
                emit_b(qc, hp, fillers.pop)
            fillers.flush()
            if n == 1:
                for ko in range(KO_G):
                    nc.sync.dma_start(out=wp_s[:, ko, :], in_=wp[:, ko, :])
        # stage 4: attention qc=3 with V t=12..15 and output projections
        # t=4..11 as fillers (V first: emit_b(3, hp) reads vaug kc=12..15)
        fillers = Fillers()
        for t in range(12, 16):
            fillers.add(lambda t=t: emit_v(t))
        for t in range(4, 12):
            for nn in (0, 1):
                fillers.add(lambda t=t, nn=nn: emit_c(t, nn))
        for hp in range(HG // 2):
            emit_b(3, hp, fillers.pop)
        fillers.flush()
        # trailing output projections for the last q-chunk
        for t in range(12, 16):
            for nn in (0, 1):
                emit_c(t, nn)
    nc.finalize()
    return nc


def _get_nc():
    if "nc" not in _cache:
        _cache["nc"] = _build()
    return _cache["nc"]


def _prep_inputs(x, W_attn, b_attn, W_proj):
    import ml_dtypes

    bfnp = ml_dtypes.bfloat16
    x = np.asarray(x, np.float32)
    W_attn = np.asarray(W_attn, np.float32)
    b_attn = np.asarray(b_attn, np.float32)
    W_proj = np.asarray(W_proj, np.float32)
    mask = (np.arange(P)[:, None] <= np.arange(P)[None, :]).astype(bfnp)
    in_maps = []
    for b in range(B):
        xTb = np.ascontiguousarray(
            x[b].T.reshape(KO_C, P, NQC, 512).transpose(1, 2, 0, 3).astype(bfnp))
        for g in range(G):
            qs, ks, vs = g * GC, C + g * GC, 2 * C + g * GC
            w2 = np.concatenate([W_attn[:, qs:qs + GC], W_attn[:, ks:ks + GC]], 1)
            # wv columns / bv reordered parity-major: (two, hp, d)
            wv_r = (W_attn[:, vs:vs + GC].reshape(C, HG // 2, 2, HD)
                    .transpose(0, 2, 1, 3).reshape(C, GC))
            bv_r = (b_attn[vs:vs + GC].reshape(HG // 2, 2, HD)
                    .transpose(1, 0, 2))
            # wp rows grouped by head pair: ko chunk = hp, [even d | odd d]
            wp_r = W_proj[g * GC:(g + 1) * GC, :]  # rows already (hp,two,d) order
            in_maps.append({
                "xT": xTb,
                "wqk": np.ascontiguousarray(
                    w2.reshape(KO_C, P, 2 * GC // P, P).transpose(1, 2, 0, 3)
                    .astype(bfnp)),
                "bqk": np.ascontiguousarray(
                    np.concatenate([b_attn[qs:qs + GC], b_attn[ks:ks + GC]])
                    .reshape(2 * GC // P, P).T),
                "wv": np.ascontiguousarray(
                    wv_r.reshape(KO_C, P, GC).transpose(1, 0, 2).astype(bfnp)),
                "bv": np.ascontiguousarray(
                    np.broadcast_to(bv_r, (P, 2, HG // 2, HD))),
                "wp": np.ascontiguousarray(
                    wp_r.reshape(KO_G, P, C).transpose(1, 0, 2)),
                "mask": mask,
            })
    return in_maps


def _run(inputs, trace=False):
    from concourse.bass_utils import run_bass_kernel_spmd

    nc = _get_nc()
    in_maps = _prep_inputs(
        inputs["x"], inputs["W_attn"], inputs["b_attn"], inputs["W_proj"]
    )
    res = run_bass_kernel_spmd(nc, in_maps, list(range(B * G)), trace=trace)
    b_proj = np.asarray(inputs["b_proj"], np.float32)
    outs = [
        res.results[2 * b]["out"] + res.results[2 * b + 1]["out"] + b_proj
        for b in range(B)
    ]
    return np.stack(outs).astype(np.float32), res


def kernel(**inputs):
    return _run(inputs, trace=False)[0]


if __name__ == "__main__":
    rng = np.random.default_rng(0)
    ins = {
        "x": rng.standard_normal((B, T, C), np.float32),
        "W_attn": rng.uniform(-0.03, 0.03, (C, 3 * C)).astype(np.float32),
        "b_attn": rng.uniform(-0.03, 0.03, (3 * C,)).astype(np.float32),
        "W_proj": rng.uniform(-0.03, 0.03, (C, C)).astype(np.float32),
        "b_proj": rng.uniform(-0.03, 0.03, (C,)).astype(np.float32),
    }
    out = kernel(**ins)
    print("ran, out shape", out.shape)
